# revision 1
# baseline (speedup 1.0000x reference)
"""Trainium2 Bass kernel for the fused attention module.

8-core sharding: data-parallel over batch (B=2) x tensor-parallel over head
groups (32 heads -> 4 groups of 8). Core c handles batch c//4, head group c%4.
Each core computes QKV projection (its head slice), RoPE, full non-causal
attention for its 8 heads, and a partial output projection against its
W_out column slice; the host sums the 4 partials per batch.

Orientation notes (PE computes out = lhsT.T @ rhs, contraction on partitions):
- qT/kT produced as [f, n] (lhsT = W slice pre-transposed on host, rhs = xT)
- v produced as [n, f] (lhsT = xT tile, rhs = WvT)
- scoresT[j, i] per head (lhsT = kT j-tile, rhs = qT i-block); softmax over j
  (partitions) is handled by a ones-column on v producing the denominator row
- RoPE rotate_half is a partition shift, done as a small matmul with a
  constant block-diagonal permutation matrix R2T
- out[i, o] partial (lhsT = attT i-tile, rhs = W_outT slice)
"""

import os
import sys

sys.path.insert(0, "/opt/trn_rl_repo")

import numpy as np

import concourse.bass as bass  # noqa: F401
import concourse.mybir as mybir
import concourse.tile as tile
from concourse import bacc
from concourse.bass import ts
from concourse.bass_utils import run_bass_kernel_spmd

F32 = mybir.dt.float32
F32R = mybir.dt.float32r
# matmul operand dtype: float32r streams 1 col/cycle (4x faster than fp32 on
# the PE) at ~tf32 precision; set ATT_DT=f32 for full fp32.
DT = F32 if os.environ.get("ATT_DT") == "f32" else F32R

P = 128
NSEQ = 2048          # sequence length
CDIM = 2048          # model dim
HD = 64              # head dim
NHC = 8              # heads per core
KT = CDIM // P       # 16 contraction tiles
NB = 256             # n-block in the fused projection phase
NNB = NSEQ // NB     # 8
IB = 512             # i-block in attention
NIB = NSEQ // IB     # 4
JT = NSEQ // P       # 16 j-tiles
FQK = 2 * NHC * HD   # 1024 qk output features per core
MF = FQK // P        # 8 f-tiles (0-3 q, 4-7 k)
EXP_FUNC = mybir.ActivationFunctionType.Exp
SCALE = 1.0 / 8.0    # 1/sqrt(HD)

_CACHED_NC = None


def _build_nc():
    nc = bacc.Bacc(None)

    xt = nc.declare_dram_parameter("xt", [NNB, P, KT, NB], DT, isOutput=False)
    wqkt = nc.declare_dram_parameter("wqkt", [MF, P, KT, P], DT, isOutput=False)
    wvt = nc.declare_dram_parameter("wvt", [P, KT, 512], DT, isOutput=False)
    wot = nc.declare_dram_parameter("wot", [P, 4, CDIM], DT, isOutput=False)
    cos2t = nc.declare_dram_parameter("cos2t", [P, NSEQ], F32, isOutput=False)
    sin2t = nc.declare_dram_parameter("sin2t", [P, NSEQ], F32, isOutput=False)
    r2t = nc.declare_dram_parameter("r2t", [P, P], DT, isOutput=False)
    onesv = nc.declare_dram_parameter("onesv", [P, JT, NHC, 1], DT, isOutput=False)
    ones1 = nc.declare_dram_parameter("ones1", [1, HD], DT, isOutput=False)
    out_part = nc.declare_dram_parameter("out_part", [NSEQ, CDIM], F32, isOutput=True)

    # DRAM spill for rotated qT/kT: [mf, p, n]
    qkspill = nc.dram_tensor("qkspill", [MF, P, NSEQ], DT)

    # ATT_REPEAT>1 duplicates the body inside one NEFF (timing experiments:
    # device-time = delta between repeat counts; results are idempotent)
    n_repeat = int(os.environ.get("ATT_REPEAT", "1"))

    with tile.TileContext(nc) as tc, nc.allow_low_precision("fp32r matmul kernel"):
        for _rep in range(n_repeat):
            _kernel_body(nc, tc, xt, wqkt, wvt, wot, cos2t, sin2t, r2t,
                         onesv, ones1, out_part, qkspill)

    nc.compile()
    return nc


def _kernel_body(nc, tc, xt, wqkt, wvt, wot, cos2t, sin2t, r2t,
                 onesv, ones1, out_part, qkspill):
        with tc.tile_pool(name="persist", bufs=1) as persist:
            ones1_sb = persist.tile([1, HD], DT)
            nc.sync.dma_start(out=ones1_sb, in_=ones1[:, :])

            # ---- Phase AB: fused v + q/k projection + RoPE. One pass over x
            # (streamed in NB-column blocks); q/k (rotated) spill to DRAM,
            # v stays resident. ----
            v1p_cm = tc.tile_pool(name="v1p", bufs=1)
            v1p = v1p_cm.__enter__()
            xtp_cm = tc.tile_pool(name="xtp", bufs=2)
            xtp = xtp_cm.__enter__()
            bconst_cm = tc.tile_pool(name="bconst", bufs=1)
            bconst = bconst_cm.__enter__()
            aconst_cm = tc.tile_pool(name="aconst", bufs=1)
            aconst = aconst_cm.__enter__()

            # Interleave wvt and xt[0] loads per k-tile so the first
            # accumulation chain starts as soon as the first k-slices land.
            wvt_sb = aconst.tile([P, KT, 512], DT)
            xt_first = xtp.tile([P, KT, NB], DT, tag="xt")
            xt0_r = xt[0]
            for kc in range(KT):
                nc.sync.dma_start(out=wvt_sb[:, kc, :], in_=wvt[:, kc, :])
                nc.sync.dma_start(out=xt_first[:, kc, :], in_=xt0_r[:, kc, :])
            v1_sb = v1p.tile([P, JT, NHC, HD + 1], DT)
            r2_sb = bconst.tile([P, P], DT)
            wqk_sb = bconst.tile([P, KT, MF, P], DT)
            nc.sync.dma_start(
                out=v1_sb[:, :, :, HD:HD + 1],
                in_=onesv[:, :, :, :],
            )
            nc.sync.dma_start(out=r2_sb, in_=r2t[:, :])
            for mf in range(MF):
                nc.sync.dma_start(
                    out=wqk_sb[:, :, mf, :],
                    in_=wqkt[mf],
                )

            with tc.tile_pool(name="csp", bufs=2) as csp, \
                 tc.tile_pool(name="stg", bufs=3) as stg, \
                 tc.tile_pool(name="psv", bufs=2, space="PSUM") as psv, \
                 tc.tile_pool(name="psqk", bufs=3, space="PSUM") as psqk, \
                 tc.tile_pool(name="psrot", bufs=2, space="PSUM") as psrot:
                for nb in range(NNB):
                    if nb == 0:
                        xt_t = xt_first
                    else:
                        xt_t = xtp.tile([P, KT, NB], DT, tag="xt")
                        nc.sync.dma_start(out=xt_t, in_=xt[nb])
                    nsl = ts(nb, NB)
                    cos_sb = csp.tile([P, NB], F32, tag="cos")
                    sin_sb = csp.tile([P, NB], F32, tag="sin")
                    nc.sync.dma_start(out=cos_sb, in_=cos2t[:, nsl])
                    nc.sync.dma_start(out=sin_sb, in_=sin2t[:, nsl])
                    # v projection for this n-block
                    for t4 in range(NB // P):
                        nt = nb * (NB // P) + t4
                        vp = psv.tile([P, 512], F32, tag="vp")
                        for kc in range(KT):
                            nc.tensor.matmul(
                                vp,
                                xt_t[:, kc, ts(t4, P)],
                                wvt_sb[:, kc, :],
                                start=(kc == 0),
                                stop=(kc == KT - 1),
                            )
                        nc.scalar.copy(
                            out=v1_sb[:, nt, :, 0:HD],
                            in_=vp.rearrange("p (h d) -> p h d", h=NHC),
                        )
                    # q/k projection + rope for this n-block
                    for mf in range(MF):
                        qp = psqk.tile([P, NB], F32, tag="qp")
                        for kc in range(KT):
                            nc.tensor.matmul(
                                qp,
                                wqk_sb[:, kc, mf, :],
                                xt_t[:, kc, :],
                                start=(kc == 0),
                                stop=(kc == KT - 1),
                            )
                        qa = stg.tile([P, NB], DT, tag="qa")
                        nc.scalar.copy(out=qa, in_=qp)
                        rp = psrot.tile([P, NB], F32, tag="rp")
                        nc.tensor.matmul(rp, r2_sb, qa, start=True, stop=True)
                        t1 = stg.tile([P, NB], F32, tag="t1")
                        nc.vector.tensor_mul(out=t1, in0=qa.bitcast(F32), in1=cos_sb)
                        t2 = stg.tile([P, NB], F32, tag="t2")
                        nc.vector.tensor_mul(out=t2, in0=rp, in1=sin_sb)
                        qk_out = stg.tile([P, NB], DT, tag="qko")
                        nc.vector.tensor_add(out=qk_out, in0=t1, in1=t2)
                        nc.gpsimd.dma_start(out=qkspill[mf, :, nsl], in_=qk_out)

            aconst_cm.__exit__(None, None, None)
            bconst_cm.__exit__(None, None, None)
            xtp_cm.__exit__(None, None, None)

            if os.environ.get("ATT_PHASE") == "ab":
                # timing-only build: stop after projections; emit a token
                # output write so the NEFF has its declared output
                dummy = v1p.tile([P, NHC, HD], F32, tag="dummy")
                nc.vector.tensor_copy(out=dummy, in_=v1_sb[:, 0, :, 0:HD].bitcast(F32))
                nc.sync.dma_start(out=out_part[0:P, 0:NHC * HD], in_=dummy.rearrange("p h d -> p (h d)"))
                v1p_cm.__exit__(None, None, None)
                return

            # ---- Phase C+D: attention per head, with the output projection
            # interleaved per i-half so its matmuls fill the PE while the
            # scalar engine (exp) is the bottleneck ----
            attp_cm = tc.tile_pool(name="attp", bufs=1)
            attp = attp_cm.__enter__()
            att_sb = attp.tile([P, 4, NSEQ], DT)
            dconst_cm = tc.tile_pool(name="dconst", bufs=1)
            dconst = dconst_cm.__enter__()
            with tc.tile_pool(name="qkp", bufs=2) as qkp, \
                 tc.tile_pool(name="expp", bufs=6) as expp, \
                 tc.tile_pool(name="smal", bufs=4) as smal, \
                 tc.tile_pool(name="evp", bufs=3) as evp, \
                 tc.tile_pool(name="pssc0", bufs=1, space="PSUM") as pssc0, \
                 tc.tile_pool(name="pssc1", bufs=1, space="PSUM") as pssc1, \
                 tc.tile_pool(name="psav", bufs=1, space="PSUM") as psav, \
                 tc.tile_pool(name="psbc", bufs=1, space="PSUM") as psbc, \
                 tc.tile_pool(name="psd", bufs=2, space="PSUM") as psd:
                # first pair's q/k ahead of the (larger) wot load
                qt0 = qkp.tile([P, NSEQ], DT, tag="qt")
                nc.sync.dma_start(out=qt0, in_=qkspill[0])
                kt0 = qkp.tile([P, NSEQ], DT, tag="kt")
                nc.sync.dma_start(out=kt0, in_=qkspill[4])
                wot_sb = dconst.tile([P, 4, CDIM], DT)
                for ct in range(4):
                    nc.sync.dma_start(out=wot_sb[:, ct, :], in_=wot[:, ct, :])
                for ib in range(NIB):
                    isl = ts(ib, IB)
                    for t in range(4):
                        if ib == 0 and t == 0:
                            qt_sb, kt_sb = qt0, kt0
                        else:
                            qt_sb = qkp.tile([P, NSEQ], DT, tag="qt")
                            nc.sync.dma_start(out=qt_sb, in_=qkspill[t])
                            kt_sb = qkp.tile([P, NSEQ], DT, tag="kt")
                            nc.sync.dma_start(out=kt_sb, in_=qkspill[4 + t])
                        # scores for BOTH heads of the pair, interleaved so the
                        # two K=64 matmuls run concurrently in the PE array's
                        # two row halves (lhsT base partitions 0 and 64)
                        expq = ([], [])
                        for grp in range(8):
                            qi, qs = divmod(grp, 2)
                            if qs == 0:
                                for h2 in range(2):
                                    exp_q = expp.tile([P, 4, IB], DT, tag="exp")
                                    expq[h2].append(exp_q)
                            sc0 = pssc0.tile([P, 2, IB], F32, tag="sc0")
                            sc1 = pssc1.tile([P, 2, IB], F32, tag="sc1")
                            for j2 in range(2):
                                jt = grp * 2 + j2
                                nc.tensor.matmul(
                                    sc0[:, j2, :],
                                    kt_sb[0:HD, ts(jt, P)],
                                    qt_sb[0:HD, isl],
                                    start=True,
                                    stop=True,
                                )
                                nc.tensor.matmul(
                                    sc1[:, j2, :],
                                    kt_sb[HD:P, ts(jt, P)],
                                    qt_sb[HD:P, isl],
                                    start=True,
                                    stop=True,
                                )
                            nc.scalar.activation(
                                out=expq[0][qi][:, ts(qs, 2), :],
                                in_=sc0,
                                func=EXP_FUNC,
                                scale=SCALE,
                            )
                            nc.scalar.activation(
                                out=expq[1][qi][:, ts(qs, 2), :],
                                in_=sc1,
                                func=EXP_FUNC,
                                scale=SCALE,
                            )
                        # attn @ v (ones-column denominator row) + normalize
                        for h2 in range(2):
                            hb = HD * h2
                            h = 2 * t + h2
                            av = psav.tile([HD + 1, 512], F32, tag="av")
                            for jt in range(JT):
                                nc.tensor.matmul(
                                    av,
                                    v1_sb[:, jt, h, :],
                                    expq[h2][jt // 4][:, jt % 4, :],
                                    start=(jt == 0),
                                    stop=(jt == JT - 1),
                                )
                            rd = smal.tile([1, IB], DT, tag="rd")
                            nc.vector.reciprocal(out=rd, in_=av[HD:HD + 1, :])
                            bc = psbc.tile([HD, IB], F32, tag="bc")
                            nc.tensor.matmul(bc, ones1_sb, rd, start=True, stop=True)
                            bc_sb = smal.tile([HD, IB], F32, tag="bcs")
                            nc.vector.tensor_copy(out=bc_sb, in_=bc)
                            nc.vector.tensor_mul(
                                out=att_sb[hb:hb + HD, t, isl],
                                in0=av[0:HD, :],
                                in1=bc_sb,
                            )
                    # output projection for this i-block (att columns complete)
                    if os.environ.get("ATT_PHASE") == "abc":
                        continue
                    for it in range(4 * ib, 4 * ib + 4):
                        for ob in range(4):
                            op = psd.tile([P, 512], F32, tag="op")
                            for ct in range(4):
                                nc.tensor.matmul(
                                    op,
                                    att_sb[:, ct, ts(it, P)],
                                    wot_sb[:, ct, ts(ob, 512)],
                                    start=(ct == 0),
                                    stop=(ct == 3),
                                )
                            o_sb = evp.tile([P, 512], F32, tag="osb")
                            nc.vector.tensor_copy(out=o_sb, in_=op)
                            nc.sync.dma_start(
                                out=out_part[ts(it, P), ts(ob, 512)], in_=o_sb
                            )
                if os.environ.get("ATT_PHASE") == "abc":
                    o_dummy = evp.tile([P, 512], F32, tag="osb")
                    nc.vector.tensor_copy(out=o_dummy, in_=att_sb[:, 0, 0:512].bitcast(F32))
                    nc.sync.dma_start(out=out_part[0:P, 0:512], in_=o_dummy)
            dconst_cm.__exit__(None, None, None)
            attp_cm.__exit__(None, None, None)
            v1p_cm.__exit__(None, None, None)


def _rot_matrix():
    r = np.zeros((HD, HD), dtype=np.float32)
    for d in range(32):
        r[d, d + 32] = -1.0
    for d in range(32, HD):
        r[d, d - 32] = 1.0
    r2 = np.zeros((P, P), dtype=np.float32)
    r2[0:HD, 0:HD] = r
    r2[HD:P, HD:P] = r
    return np.ascontiguousarray(r2.T)


def _core_inputs(x, cos_t, sin_t, W_qkv, W_out, core):
    b, g = divmod(core, 4)
    hs = g * NHC * HD  # feature offset of this head group (512 per group)

    xT = np.ascontiguousarray(x[b].T)  # [c, n]
    xt = np.ascontiguousarray(
        xT.reshape(KT, P, NNB, NB).transpose(2, 1, 0, 3)
    )

    Wq = W_qkv[hs:hs + 512]
    Wk = W_qkv[CDIM + hs:CDIM + hs + 512]
    Wv = W_qkv[2 * CDIM + hs:2 * CDIM + hs + 512]
    WqkT = np.ascontiguousarray(np.concatenate([Wq, Wk], axis=0).T)  # [c, 1024]
    wqkt = np.ascontiguousarray(
        WqkT.reshape(KT, P, MF, P).transpose(2, 1, 0, 3)
    )
    WvT = np.ascontiguousarray(Wv.T)  # [c, 512]
    wvt = np.ascontiguousarray(WvT.reshape(KT, P, 512).transpose(1, 0, 2))
    WoT = np.ascontiguousarray(W_out[:, hs:hs + 512].T)  # [c-slice 512, o 2048]
    wot = np.ascontiguousarray(WoT.reshape(4, P, CDIM).transpose(1, 0, 2))

    return {
        "xt": xt,
        "wqkt": wqkt,
        "wvt": wvt,
        "wot": wot,
        "cos2t": cos_t,
        "sin2t": sin_t,
        "r2t": _ROT,
        "onesv": _ONESV,
        "ones1": _ONES1,
    }


_ROT = _rot_matrix()
_ONESV = np.ones((P, JT, NHC, 1), dtype=np.float32)
_ONES1 = np.ones((1, HD), dtype=np.float32)


def kernel(x, freqs, W_qkv, W_out):
    global _CACHED_NC
    x = np.asarray(x, dtype=np.float32)
    freqs = np.asarray(freqs, dtype=np.float32)
    W_qkv = np.asarray(W_qkv, dtype=np.float32)
    W_out = np.asarray(W_out, dtype=np.float32)

    if _CACHED_NC is None:
        _CACHED_NC = _build_nc()
    nc = _CACHED_NC

    cos_t = np.ascontiguousarray(np.tile(np.cos(freqs.T), (2, 1)))  # [128, n]
    sin_t = np.ascontiguousarray(np.tile(np.sin(freqs.T), (2, 1)))

    in_maps = [
        _core_inputs(x, cos_t, sin_t, W_qkv, W_out, core) for core in range(8)
    ]
    trace = os.environ.get("ATT_TRACE") == "1"
    res = run_bass_kernel_spmd(nc, in_maps, core_ids=list(range(8)), trace=trace)
    if trace and res.exec_time_ns is not None:
        print(f"HW exec time: {res.exec_time_ns} ns")

    out = np.empty((2, NSEQ, CDIM), dtype=np.float32)
    for b in range(2):
        acc = np.zeros((NSEQ, CDIM), dtype=np.float64)
        for g in range(4):
            acc += res.results[4 * b + g]["out_part"]
        out[b] = acc.astype(np.float32)
    return out



# revision 13
# speedup vs baseline: 1.2303x; 1.2303x over previous
"""Trainium2 Bass kernel for the fused attention module.

8-core sharding: data-parallel over batch (B=2) x tensor-parallel over head
groups (32 heads -> 4 groups of 8). Core c handles batch c//4, head group c%4.
Each core computes QKV projection (its head slice), RoPE, full non-causal
attention for its 8 heads, and a partial output projection against its
W_out column slice; the host sums the 4 partials per batch.

v2 design (vs baseline):
- all matmul operands in bf16 (same PE rate as f32r at N>=256 per the cost
  model, half the DMA/SBUF); rotary math stays f32, one bf16 rounding on the
  stored q/k
- q/k stay resident in SBUF ([128c, 8mf, 2048n] bf16) -- no DRAM spill
- attn@v computed transposed: out[i, d] via lhsT=exp[j, i-tile], rhs=v[j, 65]
  (M=128 instead of 65 -> half the PE time); the 65th v column of ones gives
  the softmax denominator; normalize = per-partition reciprocal broadcast;
  PE-transpose (identity matmul) back to [d, i] for the output projection
- phase CD is software-pipelined ("weave"): per head-pair step, the 8
  score-groups (PE) + exps (Act) are interleaved with the previous pair's
  attn@v chains and the previous i-block's output-projection pieces so the
  Activation engine (CD bottleneck ~255us of exp) never starves.

Orientation notes (PE computes out = lhsT.T @ rhs, contraction on partitions):
- qT/kT produced as [f, n] (lhsT = W slice pre-transposed on host, rhs = xT)
- v produced as [n, f] (lhsT = xT tile, rhs = WvT)
- scoresT[j, i] per head (lhsT = kT j-tile, rhs = qT i-block)
- RoPE rotate_half is a partition shift via a constant permutation matmul
"""

import os
import sys

sys.path.insert(0, "/opt/trn_rl_repo")

import numpy as np
import ml_dtypes

import concourse.bass as bass  # noqa: F401
import concourse.mybir as mybir
import concourse.tile as tile
from concourse import bacc
from concourse.bass import ts
from concourse.bass_utils import run_bass_kernel_spmd

F32 = mybir.dt.float32
F32R = mybir.dt.float32r
BF16 = mybir.dt.bfloat16
NPBF16 = ml_dtypes.bfloat16

P = 128
NSEQ = 2048          # sequence length
CDIM = 2048          # model dim
HD = 64              # head dim
NHC = 8              # heads per core
KT = CDIM // P       # 16 contraction tiles
NB = 256             # n-block in the fused projection phase
NNB = NSEQ // NB     # 8
IB = 512             # i-block in attention
NIB = NSEQ // IB     # 4
JT = NSEQ // P       # 16 j-tiles
MF = 8               # q/k feature tiles (0-3 q, 4-7 k)
EXP_FUNC = mybir.ActivationFunctionType.Exp
SCALE = 1.0 / 8.0    # 1/sqrt(HD)

_CACHED_NC = None


def _build_nc():
    nc = bacc.Bacc(None)

    xt = nc.declare_dram_parameter("xt", [NNB, P, KT, NB], BF16, isOutput=False)
    wqkt = nc.declare_dram_parameter("wqkt", [MF, P, KT, P], BF16, isOutput=False)
    wvt = nc.declare_dram_parameter("wvt", [P, KT, 512], BF16, isOutput=False)
    wot = nc.declare_dram_parameter("wot", [P, 4, CDIM], BF16, isOutput=False)
    cos2t = nc.declare_dram_parameter("cos2t", [P, NSEQ], F32, isOutput=False)
    sin2t = nc.declare_dram_parameter("sin2t", [P, NSEQ], F32, isOutput=False)
    r2t = nc.declare_dram_parameter("r2t", [P, P], F32R, isOutput=False)
    ident = nc.declare_dram_parameter("ident", [P, P], BF16, isOutput=False)
    out_part = nc.declare_dram_parameter("out_part", [NSEQ, CDIM], F32, isOutput=True)

    n_repeat = int(os.environ.get("ATT_REPEAT", "1"))

    with tile.TileContext(nc) as tc, nc.allow_low_precision("bf16 matmul kernel"):
        for _rep in range(n_repeat):
            _kernel_body(nc, tc, xt, wqkt, wvt, wot, cos2t, sin2t, r2t,
                         ident, out_part)

    nc.compile()
    return nc


def _kernel_body(nc, tc, xt, wqkt, wvt, wot, cos2t, sin2t, r2t, ident,
                 out_part):
    with tc.tile_pool(name="persist", bufs=1) as persist:
        qk_sb = persist.tile([P, MF, NSEQ], BF16)
        v1_sb = persist.tile([P, JT, NHC, HD + 1], BF16)
        att_sb = persist.tile([P, 4, NSEQ], BF16)
        wot_sb = persist.tile([P, 4, CDIM], BF16)
        id_sb = persist.tile([P, P], BF16)
        r2_sb = persist.tile([P, P], F32R)

        nc.vector.memset(v1_sb[:, :, :, HD:HD + 1], 1.0)
        nc.sync.dma_start(out=id_sb, in_=ident[:, :])
        nc.sync.dma_start(out=r2_sb, in_=r2t[:, :])

        # ---- Phase AB: fused v + q/k projection + RoPE. One pass over x
        # (streamed in NB-column blocks); rotated q/k written to the
        # resident qk_sb, v stays resident in v1_sb. ----
        with tc.tile_pool(name="abconst", bufs=1) as abconst, \
             tc.tile_pool(name="xtp", bufs=2) as xtp, \
             tc.tile_pool(name="csp", bufs=2) as csp, \
             tc.tile_pool(name="stg", bufs=3) as stg, \
             tc.tile_pool(name="psv", bufs=2, space="PSUM") as psv, \
             tc.tile_pool(name="psqk", bufs=3, space="PSUM") as psqk, \
             tc.tile_pool(name="psrot", bufs=2, space="PSUM") as psrot:
            # Interleave wvt and xt[0] loads per k-tile so the first
            # accumulation chain starts as soon as the first k-slices land.
            wvt_sb = abconst.tile([P, KT, 512], BF16)
            xt_first = xtp.tile([P, KT, NB], BF16, tag="xt")
            xt0_r = xt[0]
            for kc in range(KT):
                nc.sync.dma_start(out=wvt_sb[:, kc, :], in_=wvt[:, kc, :])
                nc.sync.dma_start(out=xt_first[:, kc, :], in_=xt0_r[:, kc, :])
            wqk_sb = abconst.tile([P, KT, MF, P], BF16)
            for mf in range(MF):
                nc.sync.dma_start(out=wqk_sb[:, :, mf, :], in_=wqkt[mf])

            for nb in range(NNB):
                if nb == 0:
                    xt_t = xt_first
                else:
                    xt_t = xtp.tile([P, KT, NB], BF16, tag="xt")
                    nc.sync.dma_start(out=xt_t, in_=xt[nb])
                nsl = ts(nb, NB)
                cos_sb = csp.tile([P, NB], F32, tag="cos")
                sin_sb = csp.tile([P, NB], F32, tag="sin")
                nc.sync.dma_start(out=cos_sb, in_=cos2t[:, nsl])
                nc.sync.dma_start(out=sin_sb, in_=sin2t[:, nsl])
                # v projection for this n-block
                for t4 in range(NB // P):
                    nt = nb * (NB // P) + t4
                    vp = psv.tile([P, 512], F32, tag="vp")
                    for kc in range(KT):
                        nc.tensor.matmul(
                            vp,
                            xt_t[:, kc, ts(t4, P)],
                            wvt_sb[:, kc, :],
                            start=(kc == 0),
                            stop=(kc == KT - 1),
                        )
                    nc.scalar.copy(
                        out=v1_sb[:, nt, :, 0:HD],
                        in_=vp.rearrange("p (h d) -> p h d", h=NHC),
                    )
                # q/k projection + rope for this n-block
                for mf in range(MF):
                    qp = psqk.tile([P, NB], F32, tag="qp")
                    for kc in range(KT):
                        nc.tensor.matmul(
                            qp,
                            wqk_sb[:, kc, mf, :],
                            xt_t[:, kc, :],
                            start=(kc == 0),
                            stop=(kc == KT - 1),
                        )
                    qa = stg.tile([P, NB], F32R, tag="qa")
                    nc.scalar.copy(out=qa, in_=qp)
                    rp = psrot.tile([P, NB], F32, tag="rp")
                    nc.tensor.matmul(rp, r2_sb, qa, start=True, stop=True)
                    t1 = stg.tile([P, NB], F32, tag="t1")
                    nc.vector.tensor_mul(out=t1, in0=qa.bitcast(F32), in1=cos_sb)
                    t2 = stg.tile([P, NB], F32, tag="t2")
                    nc.vector.tensor_mul(out=t2, in0=rp, in1=sin_sb)
                    nc.vector.tensor_add(
                        out=qk_sb[:, mf, nsl], in0=t1, in1=t2
                    )

        # ---- Phase CD: attention + output projection, software-pipelined.
        # Per (ib, t) "weave step": 8 score groups + exps for the CURRENT
        # head pair, interleaved with attn@v chains of the PREVIOUS pair and
        # output-projection pieces of completed i-blocks. ----
        for ct in range(4):
            nc.sync.dma_start(out=wot_sb[:, ct, :], in_=wot[:, ct, :])

        with tc.tile_pool(name="expp", bufs=18) as expp, \
             tc.tile_pool(name="normp", bufs=4) as normp, \
             tc.tile_pool(name="rdp", bufs=4) as rdp, \
             tc.tile_pool(name="evp", bufs=3) as evp, \
             tc.tile_pool(name="pssc0", bufs=1, space="PSUM") as pssc0, \
             tc.tile_pool(name="pssc1", bufs=1, space="PSUM") as pssc1, \
             tc.tile_pool(name="psav", bufs=2, space="PSUM") as psav, \
             tc.tile_pool(name="pstr", bufs=1, space="PSUM") as pstr, \
             tc.tile_pool(name="psd", bufs=1, space="PSUM") as psd:

            prev = None          # (t_pair, isl, expq) of the previous step
            pending_tr = []      # [(norm_tile, h2, t_pair, i_tile), ...]
            proj_pieces = []     # [(it, ob), ...] ready for output projection

            def attnv_chain(g):
                pt, pib, pexpq = prev
                h2, it2 = divmod(g, 4)
                h = 2 * pt + h2
                av = psav.tile([P, P], F32, tag="av")
                for jt in range(JT):
                    nc.tensor.matmul(
                        av[:, 0:HD + 1],
                        pexpq[jt // 2][:, jt % 2, h2, ts(it2, P)],
                        v1_sb[:, jt, h, :],
                        start=(jt == 0),
                        stop=(jt == JT - 1),
                    )
                rd = rdp.tile([P, 1], F32, tag="rd")
                nc.vector.reciprocal(out=rd, in_=av[:, HD:HD + 1])
                norm = normp.tile([P, HD], BF16, tag="norm")
                nc.vector.tensor_scalar_mul(norm, av[:, 0:HD], rd)
                pending_tr.append((norm, h2, pt, ts(4 * pib + it2, P)))

            def flush_tr():
                norm, h2, pt, pisl = pending_tr.pop(0)
                trp = pstr.tile([HD, P], BF16, tag="tr")
                nc.tensor.transpose(trp, norm, id_sb)
                nc.vector.tensor_copy(
                    out=att_sb[h2 * HD:(h2 + 1) * HD, pt, pisl],
                    in_=trp,
                )

            def proj_piece():
                it, ob = proj_pieces.pop(0)
                op = psd.tile([P, 512], F32, tag="op")
                for ct in range(4):
                    nc.tensor.matmul(
                        op,
                        att_sb[:, ct, ts(it, P)],
                        wot_sb[:, ct, ts(ob, 512)],
                        start=(ct == 0),
                        stop=(ct == 3),
                    )
                o_sb = evp.tile([P, 512], F32, tag="osb")
                nc.vector.tensor_copy(out=o_sb, in_=op)
                nc.sync.dma_start(
                    out=out_part[ts(it, P), ts(ob, 512)], in_=o_sb
                )

            steps = [(ib, t) for ib in range(NIB) for t in range(4)]
            for ib, t in steps:
                isl = ts(ib, IB)
                qts = qk_sb[:, t, :]
                kts = qk_sb[:, 4 + t, :]
                expq = []
                for g in range(8):
                    # scores for j-tiles 2g, 2g+1, both heads of the pair;
                    # two psum tiles (one per head-half) so the next group's
                    # score writes overlap this group's exp reads
                    exp_q = expp.tile([P, 2, 2, IB], BF16, tag="exp")
                    expq.append(exp_q)
                    sc0 = pssc0.tile([P, 2, IB], F32, tag="sc0")
                    sc1 = pssc1.tile([P, 2, IB], F32, tag="sc1")
                    for j2 in range(2):
                        jt = g * 2 + j2
                        nc.tensor.matmul(
                            sc0[:, j2, :],
                            kts[0:HD, ts(jt, P)],
                            qts[0:HD, isl],
                            start=True,
                            stop=True,
                        )
                        nc.tensor.matmul(
                            sc1[:, j2, :],
                            kts[HD:P, ts(jt, P)],
                            qts[HD:P, isl],
                            start=True,
                            stop=True,
                        )
                    nc.scalar.activation(
                        out=exp_q[:, :, 0, :],
                        in_=sc0,
                        func=EXP_FUNC,
                        scale=SCALE,
                    )
                    nc.scalar.activation(
                        out=exp_q[:, :, 1, :],
                        in_=sc1,
                        func=EXP_FUNC,
                        scale=SCALE,
                    )
                    # weave: transpose of the chain issued one group ago
                    # (its DVE normalize has had a full group to finish)
                    if pending_tr:
                        flush_tr()
                    # weave: attn@v chain of the previous pair
                    if prev is not None:
                        attnv_chain(g)
                    # weave: one output-projection piece at odd groups
                    if g % 2 == 1 and proj_pieces:
                        proj_piece()
                if prev is not None and prev[0] == 3:
                    # attn@v for (ib-1, t=3) just issued -> i-block ib-1 done
                    pib = ib - 1
                    proj_pieces.extend(
                        (4 * pib + tt, ob) for tt in range(4) for ob in range(4))
                prev = (t, ib, expq)

            # tail: attn@v for the last pair, remaining transposes + pieces
            for g in range(8):
                if pending_tr:
                    flush_tr()
                attnv_chain(g)
                if proj_pieces:
                    proj_piece()
            while pending_tr:
                flush_tr()
            proj_pieces.extend((4 * 3 + tt, ob) for tt in range(4)
                               for ob in range(4))
            while proj_pieces:
                proj_piece()


def _rot_matrix():
    r = np.zeros((HD, HD), dtype=np.float32)
    for d in range(32):
        r[d, d + 32] = -1.0
    for d in range(32, HD):
        r[d, d - 32] = 1.0
    r2 = np.zeros((P, P), dtype=np.float32)
    r2[0:HD, 0:HD] = r
    r2[HD:P, HD:P] = r
    return np.ascontiguousarray(r2.T)


def _core_inputs(x, cos_t, sin_t, W_qkv, W_out, core):
    b, g = divmod(core, 4)
    hs = g * NHC * HD  # feature offset of this head group (512 per group)

    xT = np.ascontiguousarray(x[b].T)  # [c, n]
    xt = np.ascontiguousarray(
        xT.reshape(KT, P, NNB, NB).transpose(2, 1, 0, 3)
    ).astype(NPBF16)

    Wq = W_qkv[hs:hs + 512]
    Wk = W_qkv[CDIM + hs:CDIM + hs + 512]
    Wv = W_qkv[2 * CDIM + hs:2 * CDIM + hs + 512]
    WqkT = np.ascontiguousarray(np.concatenate([Wq, Wk], axis=0).T)  # [c, 1024]
    wqkt = np.ascontiguousarray(
        WqkT.reshape(KT, P, MF, P).transpose(2, 1, 0, 3)
    ).astype(NPBF16)
    WvT = np.ascontiguousarray(Wv.T)  # [c, 512]
    wvt = np.ascontiguousarray(
        WvT.reshape(KT, P, 512).transpose(1, 0, 2)).astype(NPBF16)
    WoT = np.ascontiguousarray(W_out[:, hs:hs + 512].T)  # [c-slice 512, o 2048]
    wot = np.ascontiguousarray(
        WoT.reshape(4, P, CDIM).transpose(1, 0, 2)).astype(NPBF16)

    return {
        "xt": xt,
        "wqkt": wqkt,
        "wvt": wvt,
        "wot": wot,
        "cos2t": cos_t,
        "sin2t": sin_t,
        "r2t": _ROT,
        "ident": _IDENT,
    }


_ROT = _rot_matrix()
_IDENT = np.eye(P, dtype=np.float32).astype(NPBF16)


def kernel(x, freqs, W_qkv, W_out):
    global _CACHED_NC
    x = np.asarray(x, dtype=np.float32)
    freqs = np.asarray(freqs, dtype=np.float32)
    W_qkv = np.asarray(W_qkv, dtype=np.float32)
    W_out = np.asarray(W_out, dtype=np.float32)

    if _CACHED_NC is None:
        _CACHED_NC = _build_nc()
    nc = _CACHED_NC

    cos_t = np.ascontiguousarray(np.tile(np.cos(freqs.T), (2, 1)))  # [128, n]
    sin_t = np.ascontiguousarray(np.tile(np.sin(freqs.T), (2, 1)))

    in_maps = [
        _core_inputs(x, cos_t, sin_t, W_qkv, W_out, core) for core in range(8)
    ]
    trace = os.environ.get("ATT_TRACE") == "1"
    res = run_bass_kernel_spmd(nc, in_maps, core_ids=list(range(8)), trace=trace)
    if trace and res.exec_time_ns is not None:
        print(f"HW exec time: {res.exec_time_ns} ns")

    out = np.empty((2, NSEQ, CDIM), dtype=np.float32)
    for b in range(2):
        acc = np.zeros((NSEQ, CDIM), dtype=np.float64)
        for g in range(4):
            acc += res.results[4 * b + g]["out_part"]
        out[b] = acc.astype(np.float32)
    return out
